# revision 1
# baseline (speedup 1.0000x reference)
"""CandidateFinder kernel for Trainium2 (8 NeuronCores, SPMD).

Problem: for each query i (per batch), find keys j where
  lsh_match(i,j) = any of 4 LSH hash buckets agree, AND
  trie_match(i,j) = all 12 sign bits of (batch -1) features agree.
Output [B, Sq, 64] int32: if count<=64, ascending candidate indices
right-aligned with -1 padding; if count>64, ascending top-64 by dot-sim.

Device strategy: the pair predicate is one matmul + one thresholding pass.
  - one-hot encode the 4 hash ids (4*32 = 128 dims, fp8) -> a K=128 matmul
    gives lshdot = #agreeing hash buckets for a [128-key, 512-query] tile
  - the trie condition is batch-independent (signs always come from batch
    B-1) and tiny on host: precompute a per-(key, query) fp8 threshold table
    thr = 0.5 if the 12-bit sign patterns agree else 240
      match <=> lshdot >= thr   (exact: lshdot is an integer 0..4)
  - sharding: core c handles query indices c*512..(c+1)*512 for BOTH batches
    (thr shared across batches); full key set replicated.
  - per key tile: two K=128 matmuls (one per batch) -> [128,1024] f32 PSUM;
    one DVE tensor_tensor is_ge against the 0-step-broadcast thr slice ->
    fp8 mask bytes (0x38 iff match); 4 key tiles staged per SBUF tile,
    16 DMAs ship raw bytes. Host decodes bytes -> candidate indices (exact),
    right-aligns with -1 padding, and handles the (astronomically rare)
    count>64 top-k branch with an exact host fallback.
Measured: ~55 us HW exec on 8 cores (PE ~31 us busy, DVE is_ge ~38 us busy;
PE clock is capped at 1.2 GHz in this environment, DVE 0.96 GHz).
"""

import copy

import numpy as np
from ml_dtypes import bfloat16, float8_e4m3

import bass_rust
import concourse.bacc as bacc
import concourse.tile as tile
from concourse import mybir
from concourse.bass_utils import run_bass_kernel_spmd

B, S, D = 2, 4096, 12
H, BUCKETS, BW = 4, 32, 4.0
KMAX = 64
NCORES = 8
QPC = S // NCORES          # 512 query indices per core (x2 batches)
NKT = S // 128             # 32 key tiles
THRESH = 96.5
MATCH_BYTE = 0x38          # fp8e4 bit pattern of +1.0

TRACE = False              # set True (module flag) to capture an NTFF trace
LAST_RESULTS = None

_nc_cache = None


def _bcast2(ap):
    """Insert a 0-step [*, 2] dim after the partition dim (free broadcast)."""
    b = copy.copy(ap)
    b.ap = bass_rust.VecI64Pair([list(ap.ap[0]), [0, 2], list(ap.ap[1])])
    return b


def _build():
    global _nc_cache
    if _nc_cache is not None:
        return _nc_cache
    nc = bacc.Bacc()
    bf16 = mybir.dt.bfloat16
    f8 = mybir.dt.float8e4
    f32 = mybir.dt.float32

    ft_oh = nc.dram_tensor("ft_oh", [2, 128, QPC], f8, kind="ExternalInput")
    gt_oh = nc.dram_tensor("gt_oh", [2, 128, S], f8, kind="ExternalInput")
    thr_d = nc.dram_tensor("thr", [NKT // 4, 128, 4, QPC], f8, kind="ExternalInput")
    # [g8, key-in-tile, j, batch, query]
    out_d = nc.dram_tensor("out", [NKT // 4, 128, 4, 2, QPC], f8,
                           kind="ExternalOutput")

    with tile.TileContext(nc) as tc:
        with (
            tc.tile_pool(name="keys", bufs=1) as pool_k,
            tc.tile_pool(name="qrs", bufs=1) as pool_q,
            tc.tile_pool(name="msk", bufs=3) as pool_m,
            tc.tile_pool(name="ps_a", bufs=3, space="PSUM") as pool_pa,
        ):
            # loads ordered so key-tile 0 dependencies land first; bulk key
            # one-hots go through SWDGE (gpsimd) to parallelize trigger issue
            f_oh = []
            for b in range(2):
                t1 = pool_q.tile([128, QPC], f8, tag=f"foh{b}")
                nc.sync.dma_start(out=t1[:], in_=ft_oh[b])
                f_oh.append(t1)
            g_oh = [[], []]
            thr_t = []
            for i in range(8):
                for b in range(2):
                    t_ = pool_k.tile([128, 512], f8, tag=f"goh{b}_{i}")
                    nc.gpsimd.dma_start(
                        out=t_[:], in_=gt_oh[b][:, i * 512:(i + 1) * 512])
                    g_oh[b].append(t_)
                tt = pool_k.tile([128, 4 * QPC], f8, tag=f"thr{i}")
                nc.sync.dma_start(out=tt[:], in_=thr_d[i])
                thr_t.append(tt)

            msk = None
            for kt in range(NKT):
                if kt % 4 == 0:
                    msk = pool_m.tile([128, 4 * 2 * QPC], f8, tag="msk",
                                      name=f"msk_{kt}")
                thr_ap = _bcast2(
                    thr_t[kt // 4][:, (kt % 4) * QPC:(kt % 4 + 1) * QPC])
                msk_ap = msk[:, (kt % 4) * 1024:(kt % 4 + 1) * 1024]
                psA = pool_pa.tile([128, 2 * QPC], f32)
                for b in range(2):
                    nc.tensor.matmul(
                        psA[:, b * QPC:(b + 1) * QPC],
                        lhsT=g_oh[b][kt // 4][:, (kt % 4) * 128:(kt % 4 + 1) * 128],
                        rhs=f_oh[b][:],
                        start=True, stop=True,
                    )
                nc.vector.tensor_tensor(
                    msk_ap.rearrange("p (b n) -> p b n", b=2),
                    psA[:].rearrange("p (b n) -> p b n", b=2),
                    thr_ap,
                    mybir.AluOpType.is_ge,
                )
                if kt % 2 == 1:
                    h = (kt % 4) // 2
                    nc.sync.dma_start(
                        out=out_d[kt // 4][:, h * 2:(h + 1) * 2],
                        in_=msk[:, h * 2048:(h + 1) * 2048])

    nc.compile()  # wait legalization + reg alloc (bass2jax does not finalize)
    _nc_cache = nc
    return nc


def _hashes(x, proj):
    # mirror: floor((x @ lsh_proj) / BW).astype(int32) % BUCKETS
    d = x.astype(np.float32) @ proj.astype(np.float32)
    return np.floor(d / BW).astype(np.int32) % BUCKETS


def _prep(q, k, proj):
    qh = _hashes(q, proj)                       # [B,S,4]
    kh = _hashes(k, proj)
    rng = np.arange(BUCKETS, dtype=np.int32)
    q_oh = (qh[..., None] == rng).reshape(B, S, 128)
    k_oh = (kh[..., None] == rng).reshape(B, S, 128)
    sq = np.where(q[-1] > 0, np.float32(1.0), np.float32(-1.0))   # [S,12]
    sk = np.where(k[-1] > 0, np.float32(1.0), np.float32(-1.0))
    ftoh = np.ascontiguousarray(q_oh.astype(float8_e4m3).transpose(0, 2, 1))  # [B,128,S]
    gtoh = np.ascontiguousarray(k_oh.astype(float8_e4m3).transpose(0, 2, 1))
    # trie thresholds (batch-independent): thr[j, i] = 0.5 if the 12-bit sign
    # patterns of query i and key j agree else 240; match <=> lshdot >= thr
    pw = (1 << np.arange(D)).astype(np.int32)
    pat_q = ((sq > 0).astype(np.int32) @ pw).astype(np.int32)   # [S]
    pat_k = ((sk > 0).astype(np.int32) @ pw).astype(np.int32)
    eq = pat_k[:, None] == pat_q[None, :]                        # [Sk, Sq]
    b_lo = np.array(0.5, float8_e4m3).tobytes()[0]
    b_hi = np.array(240.0, float8_e4m3).tobytes()[0]
    thr = np.where(eq, np.uint8(b_lo), np.uint8(b_hi)).view(float8_e4m3)
    return qh, kh, sq, sk, ftoh, gtoh, thr


def _mask_row(b, i, qh, kh, sq, sk):
    lsh = (qh[b, i][None, :] == kh[b]).any(-1)                  # [S]
    trie = (sq[i][None, :] == sk).all(-1)                       # [S]
    return lsh & trie


def _topk_row(q, k, b, i, maskrow):
    sims = q[b, i].astype(np.float32) @ k[b].astype(np.float32).T
    vals = np.where(maskrow, sims, -np.inf)
    top = np.argsort(-vals, kind="stable")[:KMAX]               # jax top_k tiebreak
    return np.sort(top).astype(np.int32)


def _ensure_ntff_hook():
    """The container's antenv stub lacks axon_hooks; synthesize it from the
    boot module's ctypes NTFF helper so trace=True can capture HW timings."""
    import sys
    import types
    try:
        from antenv.axon_hooks import get_axon_ntff_profile_hook  # noqa: F401
        return
    except ImportError:
        pass
    from trn_agent_boot.trn_boot import _ntff_profile_via_ctypes
    hook = _ntff_profile_via_ctypes("/opt/axon/libaxon_pjrt.so")
    mod = types.ModuleType("antenv.axon_hooks")
    state = {"hook": hook}
    mod.get_axon_ntff_profile_hook = lambda: state["hook"]
    mod.set_axon_ntff_profile_hook = lambda h: state.update(hook=h)
    import antenv
    antenv.axon_hooks = mod
    sys.modules["antenv.axon_hooks"] = mod


def kernel(**inputs):
    global LAST_RESULTS
    q = np.asarray(inputs["query_features_up"], np.float32)
    k = np.asarray(inputs["key_features_up"], np.float32)
    proj = np.asarray(inputs["lsh_proj"], np.float32)

    qh, kh, sq, sk, ftoh, gtoh, thr = _prep(q, k, proj)

    nc = _build()
    in_maps = []
    for c in range(NCORES):
        qoff = c * QPC
        in_maps.append({
            "ft_oh": np.ascontiguousarray(ftoh[:, :, qoff:qoff + QPC]),
            "gt_oh": gtoh,
            "thr": np.ascontiguousarray(
                thr[:, qoff:qoff + QPC]
                .reshape(NKT // 4, 4, 128, QPC).transpose(0, 2, 1, 3)),
        })
    if TRACE:
        _ensure_ntff_hook()
    res = run_bass_kernel_spmd(
        nc, in_maps, core_ids=list(range(NCORES)), trace=TRACE
    )
    LAST_RESULTS = res

    # raw mask bytes -> bool match grid [B, Sq, Sk]
    match = np.empty((B, S, S), np.bool_)
    for c in range(NCORES):
        raw = res.results[c]["out"].view(np.uint8)   # [8, 128, 4, 2, QPC]
        # key = (g8*4 + j)*128 + p ; query = c*QPC + n
        m = (raw == MATCH_BYTE).transpose(3, 4, 0, 2, 1)  # [b, n, g8, j, p]
        match[:, c * QPC:(c + 1) * QPC, :] = m.reshape(2, QPC, S)

    cb, cq, ci = np.nonzero(match)
    rowid = cb.astype(np.int64) * S + cq
    counts = np.bincount(rowid, minlength=B * S)
    starts = np.concatenate(([0], np.cumsum(counts)))[:-1]
    ranks = np.arange(len(ci)) - starts[rowid]

    out = np.full((B * S, KMAX), -1, np.int32)
    cnt_row = counts[rowid]
    ok = cnt_row <= KMAX
    out[rowid[ok], (KMAX - cnt_row + ranks)[ok]] = ci[ok]

    # exact host fallback for count > KMAX rows (never happens in practice)
    for r in np.nonzero(counts > KMAX)[0]:
        b, i = divmod(int(r), S)
        mrow = _mask_row(b, i, qh, kh, sq, sk)
        out[r] = _topk_row(q, k, b, i, mrow)

    return out.reshape(B, S, KMAX)



# revision 4
# speedup vs baseline: 1.2756x; 1.2756x over previous
"""CandidateFinder kernel for Trainium2 (8 NeuronCores, SPMD).

Problem: for each query i (per batch), find keys j where
  lsh_match(i,j) = any of 4 LSH hash buckets agree, AND
  trie_match(i,j) = all 12 sign bits of (batch -1) features agree.
Output [B, Sq, 64] int32: if count<=64, ascending candidate indices
right-aligned with -1 padding; if count>64, ascending top-64 by dot-sim.

Device strategy (v2): one matmul + one constant-threshold pass per pair.
  - The gaussian inputs only populate a handful of the 32 LSH buckets per
    hash (~30 distinct buckets total across the 4 hashes). Host remaps each
    hash's occurring bucket values to a compact one-hot (30 dims) and
    appends the 12 trie sign dims (keys sgn in {-1,+1}, queries 2*sgn in
    {-2,+2}), zero-padded to K=64:
      s = lshdot + 2*signdot,  match <=> s >= 24.5
    (signdot=12 gives s = 24+lshdot, so s>=25 iff any hash agrees;
     signdot<=10 gives s <= 20+4 = 24.)  All values exact in fp8/f32.
  - Batch 0's encoding lives in partitions 0..63, batch 1's in 64..127, so
    the two per-key-tile matmuls occupy disjoint PE row groups and run
    concurrently (row-tiled K=64).
  - The threshold pass splits each [128, 1024] PSUM tile between DVE
    (tensor_scalar is_ge -> fp8 0/1) and ACT (Relu(s-24.5) -> fp8,
    nonzero iff match), the two fastest PSUM-reading engines.
  - Mask bytes ship to HBM; host decodes candidate indices exactly,
    right-aligns with -1 padding, and handles the (astronomically rare)
    count>64 top-k branch with an exact host fallback.
"""

import numpy as np
from ml_dtypes import float8_e4m3

import concourse.bacc as bacc
import concourse.tile as tile
from concourse import mybir
from concourse.bass_utils import run_bass_kernel_spmd

B, S, D = 2, 4096, 12
H, BUCKETS, BW = 4, 32, 4.0
KMAX = 64
NCORES = 8
QPC = S // NCORES          # 512 query indices per core (x2 batches)
NKT = S // 128             # 32 key tiles
KDIM = 64                  # padded contraction dims per batch (<=64 required)
SPLIT = 456                # DVE gets cols [0:SPLIT), ACT gets [SPLIT:1024)
THRESH = 24.5

TRACE = False              # set True (module flag) to capture an NTFF trace
LAST_RESULTS = None

_nc_cache = None


def _build():
    global _nc_cache
    if _nc_cache is not None:
        return _nc_cache
    nc = bacc.Bacc()
    f8 = mybir.dt.float8e4
    f32 = mybir.dt.float32

    ft_d = nc.dram_tensor("ft", [128, QPC], f8, kind="ExternalInput")
    gt_d = nc.dram_tensor("gt", [128, S], f8, kind="ExternalInput")
    # [chunk of 2 key tiles, key-in-tile, (kt-in-chunk, batch, query)]
    out_d = nc.dram_tensor("out", [NKT // 2, 128, 2048], f8,
                           kind="ExternalOutput")

    with tile.TileContext(nc) as tc:
        with (
            tc.tile_pool(name="keys", bufs=1) as pool_k,
            tc.tile_pool(name="qrs", bufs=1) as pool_q,
            tc.tile_pool(name="msk", bufs=3) as pool_m,
            tc.tile_pool(name="ps_a", bufs=3, space="PSUM") as pool_pa,
        ):
            bias_t = pool_q.tile([128, 1], f32, tag="bias")
            nc.gpsimd.memset(bias_t[:], -THRESH)
            f_t = pool_q.tile([128, QPC], f8, tag="ft")
            nc.sync.dma_start(out=f_t[:], in_=ft_d[:])
            g_t = pool_k.tile([128, S], f8, tag="gt")
            for c in range(4):
                nc.sync.dma_start(
                    out=g_t[:, c * 1024:(c + 1) * 1024],
                    in_=gt_d[:, c * 1024:(c + 1) * 1024])

            msk = None
            for kt in range(NKT):
                if kt % 2 == 0:
                    msk = pool_m.tile([128, 2048], f8, tag="msk",
                                      name=f"msk_{kt}")
                psA = pool_pa.tile([128, 2 * QPC], f32)
                for b in range(2):
                    nc.tensor.matmul(
                        psA[:, b * QPC:(b + 1) * QPC],
                        lhsT=g_t[b * KDIM:(b + 1) * KDIM,
                                 kt * 128:(kt + 1) * 128],
                        rhs=f_t[b * KDIM:(b + 1) * KDIM, :],
                        start=True, stop=True,
                    )
                moff = (kt % 2) * 1024
                nc.vector.tensor_scalar(
                    msk[:, moff:moff + SPLIT],
                    psA[:, 0:SPLIT],
                    THRESH, None,
                    mybir.AluOpType.is_ge,
                )
                nc.scalar.activation(
                    msk[:, moff + SPLIT:moff + 1024],
                    psA[:, SPLIT:1024],
                    mybir.ActivationFunctionType.Relu,
                    bias=bias_t[:], scale=1.0,
                )
                if kt % 2 == 1:
                    nc.sync.dma_start(out=out_d[kt // 2], in_=msk[:])

    nc.compile()  # wait legalization + reg alloc (bass2jax does not finalize)
    _nc_cache = nc
    return nc


def _hashes(x, proj):
    # mirror: floor((x @ lsh_proj) / BW).astype(int32) % BUCKETS
    d = x.astype(np.float32) @ proj.astype(np.float32)
    return np.floor(d / BW).astype(np.int32) % BUCKETS


def _prep(q, k, proj):
    qh = _hashes(q, proj)                       # [B,S,4]
    kh = _hashes(k, proj)
    sq = np.where(q[-1] > 0, np.float32(1.0), np.float32(-1.0))   # [S,12]
    sk = np.where(k[-1] > 0, np.float32(1.0), np.float32(-1.0))

    # Compact per-hash bucket remap: only values that actually occur get a
    # one-hot slot.  offs[h] = base row of hash h's block.
    luts, offs, base = [], [], 0
    for h in range(H):
        vals = np.unique(np.concatenate(
            [qh[:, :, h].ravel(), kh[:, :, h].ravel()]))
        lut = np.full(BUCKETS, -1, np.int32)
        lut[vals] = np.arange(len(vals), dtype=np.int32)
        luts.append(lut)
        offs.append(base)
        base += len(vals)
    n_oh = base
    kdim = n_oh + D                             # used contraction dims
    if kdim > KDIM:
        return qh, kh, sq, sk, None, None, kdim

    # encodings: [128, n] fp8 with batch b in partition rows b*KDIM..
    def encode(hsh, sgn, sign_scale):
        n = hsh.shape[1]
        enc = np.zeros((128, n), np.float32)
        for b in range(B):
            r0 = b * KDIM
            for h in range(H):
                slot = luts[h][hsh[b, :, h]] + offs[h]   # [n], all >= 0
                enc[r0 + slot, np.arange(n)] = 1.0
            enc[r0 + n_oh:r0 + n_oh + D, :] = sign_scale * sgn.T
        return enc.astype(float8_e4m3)

    ft = encode(qh, sq, 2.0)                    # [128, S] queries
    gt = encode(kh, sk, 1.0)                    # [128, S] keys
    return qh, kh, sq, sk, ft, gt, kdim


def _mask_row(b, i, qh, kh, sq, sk):
    lsh = (qh[b, i][None, :] == kh[b]).any(-1)                  # [S]
    trie = (sq[i][None, :] == sk).all(-1)                       # [S]
    return lsh & trie


def _topk_row(q, k, b, i, maskrow):
    sims = q[b, i].astype(np.float32) @ k[b].astype(np.float32).T
    vals = np.where(maskrow, sims, -np.inf)
    top = np.argsort(-vals, kind="stable")[:KMAX]               # jax top_k tiebreak
    return np.sort(top).astype(np.int32)


def _pack(match, q, k, qh, kh, sq, sk):
    """bool match grid [B, Sq, Sk] -> output [B, S, KMAX] int32."""
    cb, cq, ci = np.nonzero(match)
    rowid = cb.astype(np.int64) * S + cq
    counts = np.bincount(rowid, minlength=B * S)
    starts = np.concatenate(([0], np.cumsum(counts)))[:-1]
    ranks = np.arange(len(ci)) - starts[rowid]

    out = np.full((B * S, KMAX), -1, np.int32)
    cnt_row = counts[rowid]
    ok = cnt_row <= KMAX
    out[rowid[ok], (KMAX - cnt_row + ranks)[ok]] = ci[ok]

    # exact host fallback for count > KMAX rows (never happens in practice)
    for r in np.nonzero(counts > KMAX)[0]:
        b, i = divmod(int(r), S)
        mrow = _mask_row(b, i, qh, kh, sq, sk)
        out[r] = _topk_row(q, k, b, i, mrow)

    return out.reshape(B, S, KMAX)


def _ensure_ntff_hook():
    """The container's antenv stub lacks axon_hooks; synthesize it from the
    boot module's ctypes NTFF helper so trace=True can capture HW timings."""
    import sys
    import types
    try:
        from antenv.axon_hooks import get_axon_ntff_profile_hook  # noqa: F401
        return
    except ImportError:
        pass
    from trn_agent_boot.trn_boot import _ntff_profile_via_ctypes
    hook = _ntff_profile_via_ctypes("/opt/axon/libaxon_pjrt.so")
    mod = types.ModuleType("antenv.axon_hooks")
    state = {"hook": hook}
    mod.get_axon_ntff_profile_hook = lambda: state["hook"]
    mod.set_axon_ntff_profile_hook = lambda h: state.update(hook=h)
    import antenv
    antenv.axon_hooks = mod
    sys.modules["antenv.axon_hooks"] = mod


def kernel(**inputs):
    global LAST_RESULTS
    q = np.asarray(inputs["query_features_up"], np.float32)
    k = np.asarray(inputs["key_features_up"], np.float32)
    proj = np.asarray(inputs["lsh_proj"], np.float32)

    qh, kh, sq, sk, ft, gt, kdim = _prep(q, k, proj)
    if ft is None:
        # pathological bucket spread (never with gaussian data): exact host path
        lsh = (qh[:, :, None, :] == kh[:, None, :, :]).any(-1)
        trie = (sq[:, None, :] == sk[None, :, :]).all(-1)
        return _pack(lsh & trie[None], q, k, qh, kh, sq, sk)

    nc = _build()
    in_maps = []
    for c in range(NCORES):
        qoff = c * QPC
        in_maps.append({
            "ft": np.ascontiguousarray(ft[:, qoff:qoff + QPC]),
            "gt": gt,
        })
    if TRACE:
        _ensure_ntff_hook()
    res = run_bass_kernel_spmd(
        nc, in_maps, core_ids=list(range(NCORES)), trace=TRACE
    )
    LAST_RESULTS = res

    # raw mask bytes -> bool match grid [B, Sq, Sk]
    match = np.empty((B, S, S), np.bool_)
    for c in range(NCORES):
        raw = res.results[c]["out"].view(np.uint8)   # [16, 128, 2048]
        # col = (j * 2 + b) * 512 + n ; key = (g*2 + j)*128 + p
        m = (raw & 0x7F).reshape(16, 128, 2, 2, QPC) != 0   # [g, p, j, b, n]
        match[:, c * QPC:(c + 1) * QPC, :] = (
            m.transpose(3, 4, 0, 2, 1).reshape(2, QPC, S))
    return _pack(match, q, k, qh, kh, sq, sk)
